# revision 16
# baseline (speedup 1.0000x reference)
"""MultiHeadMlp TRN2 kernel: grouped per-head MLP + SE channel attention.

Full-input contract: kernel(**inputs) takes the complete arrays and returns
the complete output. Internally shards data-parallel over the batch dim
(B=8 -> 8 NeuronCores), builds one SPMD Bass/Tile program, and runs it via
run_bass_kernel_spmd.

Math (per batch element b, all tokens local to one core):
    xh = x.reshape(N, H, D)
    h  = gelu(xh @ W1 + b1)          per head, D=256 -> HID=1024
    o  = h @ W2 + b2                 per head, HID   -> D
    out = concat_heads(o)            (N, C)
    pooled = out.mean(axis=0)        (C,)
    gate = sigmoid(relu(pooled@cw1+cb1)@cw2+cb2)
    y = out * (1 + gate)

Layout strategy: everything on-chip is channel-major ("transposed"):
the host hands the kernel x^T (and un-transposes y^T on the way out), so
W1 [D,HID] / W2 [HID,D] serve directly as matmul lhsT operands, the SE
pool is a free-dim reduction, the gate is a native per-partition scalar
multiply, and the device never transposes anything.
"""

import numpy as np
import ml_dtypes

B = 8
N = 4096
DIM = 1024
H = 4
HD = 256           # head dim
HID = 1024         # per-head hidden
SQ = 64            # squeeze dim
TCH = 512          # tokens per chunk
NCHUNK = N // TCH  # 8
NCORES = 8

_BF = ml_dtypes.bfloat16

_cache = {}


def _build():
    from contextlib import ExitStack

    import concourse.bass as bass
    import concourse.mybir as mybir
    from concourse import bacc
    from concourse.tile import TileContext

    dt = mybir.dt
    bf = dt.bfloat16
    f32 = dt.float32
    Act = mybir.ActivationFunctionType
    Alu = mybir.AluOpType
    Ax = mybir.AxisListType

    nc = bacc.Bacc("TRN2", target_bir_lowering=False, debug=False)

    xt = nc.dram_tensor("xt", [DIM, N], bf, kind="ExternalInput")
    w1 = nc.dram_tensor("w1", [H, HD, HID], bf, kind="ExternalInput")
    w2 = nc.dram_tensor("w2", [H, HID, HD], bf, kind="ExternalInput")
    b1t = nc.dram_tensor("b1t", [128, H * 8], f32, kind="ExternalInput")
    b2t = nc.dram_tensor("b2t", [128, 8], f32, kind="ExternalInput")
    cw1 = nc.dram_tensor("cw1", [DIM, SQ], bf, kind="ExternalInput")
    cb1t = nc.dram_tensor("cb1t", [SQ, 1], f32, kind="ExternalInput")
    cw2 = nc.dram_tensor("cw2", [SQ, DIM], bf, kind="ExternalInput")
    cb2t = nc.dram_tensor("cb2t", [128, 8], f32, kind="ExternalInput")
    outT = nc.dram_tensor("outT", [DIM, N], bf, kind="ExternalOutput")

    with TileContext(nc) as tc, ExitStack() as ctx:
        const = ctx.enter_context(tc.tile_pool(name="const", bufs=1))
        hpool = ctx.enter_context(tc.tile_pool(name="hpool", bufs=2))
        pg1 = ctx.enter_context(tc.tile_pool(name="pg1", bufs=3, space="PSUM"))
        pg2 = ctx.enter_context(tc.tile_pool(name="pg2", bufs=3, space="PSUM"))

        # ---- activation-table warmup (overlaps the load phase) ----
        warm = const.tile([128, 1], f32, name="warm", tag="warm")
        nc.vector.memset(warm, 0.0)
        nc.scalar.activation(out=warm, in_=warm, func=Act.Sigmoid)
        nc.scalar.activation(out=warm, in_=warm, func=Act.Relu)
        nc.scalar.activation(out=warm, in_=warm, func=Act.Gelu)

        # ---- weights + x^T, ordered by first use, all on HWDGE ----
        # consolidated per-head weight tiles: one DMA each
        w1sb = [const.tile([128, 2, HID], bf, name=f"w1sb_{h}",
                           tag=f"w1sb_{h}") for h in range(H)]
        w2sb = [const.tile([128, 8, HD], bf, name=f"w2sb_{h}",
                           tag=f"w2sb_{h}") for h in range(H)]
        xfull = [const.tile([128, N], bf, name=f"xfull_{c}",
                            tag=f"xfull_{c}") for c in range(8)]
        b1sb = const.tile([128, H * 8], f32, name="b1sb", tag="b1sb")
        b2sb = const.tile([128, 8], f32, name="b2sb", tag="b2sb")

        HN = N // 2
        nc.sync.dma_start(out=w1sb[0],
                          in_=w1[0].rearrange("(k p) n -> p k n", p=128))
        nc.sync.dma_start(out=b1sb, in_=b1t[:, :])
        # first chunk's x slices first for the earliest possible PE start
        nc.sync.dma_start(out=xfull[0][:, :TCH], in_=xt[0:128, :TCH])
        nc.sync.dma_start(out=xfull[1][:, :TCH], in_=xt[128:256, :TCH])
        nc.sync.dma_start(out=xfull[0][:, TCH:HN], in_=xt[0:128, TCH:HN])
        nc.sync.dma_start(out=xfull[1][:, TCH:HN], in_=xt[128:256, TCH:HN])
        nc.sync.dma_start(out=w2sb[0],
                          in_=w2[0].rearrange("(k p) n -> p k n", p=128))
        nc.sync.dma_start(out=b2sb, in_=b2t[:, :])
        for h in range(1, H):
            nc.sync.dma_start(out=w1sb[h],
                              in_=w1[h].rearrange("(k p) n -> p k n", p=128))
            nc.sync.dma_start(out=xfull[2 * h][:, :HN],
                              in_=xt[h * 256:h * 256 + 128, :HN])
            nc.sync.dma_start(out=xfull[2 * h + 1][:, :HN],
                              in_=xt[h * 256 + 128:(h + 1) * 256, :HN])
            nc.sync.dma_start(out=w2sb[h],
                              in_=w2[h].rearrange("(k p) n -> p k n", p=128))
        for c in range(8):
            nc.sync.dma_start(out=xfull[c][:, HN:],
                              in_=xt[c * 128:(c + 1) * 128, HN:])
        cw1sb = const.tile([128, 8, SQ], bf, name="cw1sb", tag="cw1sb")
        nc.sync.dma_start(out=cw1sb,
                          in_=cw1.rearrange("(c p) n -> p c n", p=128))
        cb1sb = const.tile([SQ, 1], f32, name="cb1sb", tag="cb1sb")
        nc.sync.dma_start(out=cb1sb, in_=cb1t[:, :])
        cw2sb = const.tile([SQ, DIM], bf, name="cw2sb", tag="cw2sb")
        nc.sync.dma_start(out=cw2sb, in_=cw2[:, :])
        cb2sb = const.tile([128, 8], f32, name="cb2sb", tag="cb2sb")
        nc.sync.dma_start(out=cb2sb, in_=cb2t[:, :])

        # channel-major out accumulator (persists across whole kernel)
        oT = []
        for c in range(8):
            t = const.tile([128, N], bf, name=f"oT_{c}", tag=f"oT_{c}")
            oT.append(t)
        # per-(chunk, chan-tile) row sums for the SE pool
        prow = const.tile([128, NCHUNK * 8], f32, name="prow", tag="prow")

        # ---- main loop over token chunks ----
        for i in range(NCHUNK):
            t0 = i * TCH
            for h in range(H):
                # GEMM1: h^T[m-tile] = gelu(W1_h^T x^T + b1)
                ht = []
                for m in range(8):
                    p1 = pg1.tile([128, TCH], f32, name="p1", tag="p1")
                    nc.tensor.matmul(
                        p1, lhsT=w1sb[h][:, 0, m * 128:(m + 1) * 128],
                        rhs=xfull[2 * h][:, t0:t0 + TCH],
                        start=True, stop=False)
                    nc.tensor.matmul(
                        p1, lhsT=w1sb[h][:, 1, m * 128:(m + 1) * 128],
                        rhs=xfull[2 * h + 1][:, t0:t0 + TCH],
                        start=False, stop=True)
                    hm = hpool.tile([128, TCH], bf, name=f"ht_{m}",
                                    tag=f"ht_{m}")
                    nc.scalar.activation(
                        out=hm, in_=p1, func=Act.Gelu,
                        bias=b1sb[:, h * 8 + m:h * 8 + m + 1])
                    ht.append(hm)
                # GEMM2: o^T[d-half] = W2_h^T h^T + b2
                for d in range(2):
                    c = h * 2 + d
                    p2 = pg2.tile([128, TCH], f32, name="p2", tag="p2")
                    for k in range(8):
                        nc.tensor.matmul(
                            p2, lhsT=w2sb[h][:, k, d * 128:(d + 1) * 128],
                            rhs=ht[k], start=(k == 0), stop=(k == 7))
                    nc.vector.tensor_scalar(
                        out=oT[c][:, t0:t0 + TCH], in0=p2,
                        scalar1=b2sb[:, c:c + 1],
                        scalar2=0.0, op0=Alu.add, op1=Alu.add,
                        accum_out=prow[:, i * 8 + c:i * 8 + c + 1])

        # ---- SE channel attention on pooled means (all channel-major) ----
        # partial reduction over chunks 0..6 runs as soon as those chunks'
        # row sums exist (overlaps chunk 7 compute); only the final add is
        # on the critical path.
        pooled_part = const.tile([128, 8], f32, name="pooled_part",
                                 tag="pooled_part")
        pooled_raw = const.tile([128, 8], f32, name="pooled_raw",
                                tag="pooled_raw")
        prow3 = prow.rearrange("p (i c) -> p i c", c=8)
        for c in range(8):
            nc.vector.tensor_reduce(
                out=pooled_part[:, c:c + 1], in_=prow3[:, 0:NCHUNK - 1, c],
                axis=Ax.X, op=Alu.add)
        nc.vector.tensor_tensor(out=pooled_raw, in0=pooled_part,
                                in1=prow3[:, NCHUNK - 1, :], op=Alu.add)
        pooledT = const.tile([128, 8], bf, name="pooledT", tag="pooledT")
        nc.vector.tensor_scalar_mul(pooledT, pooled_raw, 1.0 / N)

        pz = pg1.tile([SQ, 1], f32, name="pz", tag="p1")
        for c in range(8):
            nc.tensor.matmul(pz, lhsT=cw1sb[:, c, :], rhs=pooledT[:, c:c + 1],
                             start=(c == 0), stop=(c == 7))
        z1sb = const.tile([SQ, 1], bf, name="z1sb", tag="z1sb")
        nc.scalar.activation(out=z1sb, in_=pz, func=Act.Relu, bias=cb1sb)

        # gate^T[c] = 1 + sigmoid(cw2^T z1 + cb2), per chan-tile
        g1T = const.tile([128, 8], f32, name="g1T", tag="g1T")
        for c in range(8):
            gp = pg2.tile([128, 1], f32, name="gp", tag="p2")
            nc.tensor.matmul(gp, lhsT=cw2sb[:, c * 128:(c + 1) * 128],
                             rhs=z1sb, start=True, stop=True)
            nc.scalar.activation(out=g1T[:, c:c + 1], in_=gp,
                                 func=Act.Sigmoid, bias=cb2sb[:, c:c + 1])
        nc.vector.tensor_scalar_add(g1T, g1T, 1.0)

        # ---- final scale + store (in-place on oT; DVE with GpSimd assist) ----
        for c in range(8):
            for half in range(2):
                sl = slice(half * 2048, (half + 1) * 2048)
                if (c, half) in ((0, 0), (1, 0), (2, 0), (3, 0)):
                    nc.gpsimd.tensor_scalar_mul(
                        oT[c][:, sl], oT[c][:, sl], g1T[:, c:c + 1])
                else:
                    nc.vector.tensor_scalar_mul(
                        oT[c][:, sl], oT[c][:, sl], g1T[:, c:c + 1])
                nc.sync.dma_start(out=outT[c * 128:(c + 1) * 128, sl],
                                  in_=oT[c][:, sl])

    nc.compile()
    return nc


def _get_nc():
    if "nc" not in _cache:
        _cache["nc"] = _build()
    return _cache["nc"]


def _make_in_maps(x, W1, b1, W2, b2, cw1, cb1, cw2, cb2):
    # bf16 + pre-transposed x: (B, N, DIM) -> per-core (DIM, N)
    xb = np.asarray(x, dtype=_BF)
    w1b = np.asarray(W1, dtype=_BF)
    w2b = np.asarray(W2, dtype=_BF)
    cw1b = np.asarray(cw1, dtype=_BF)
    cw2b = np.asarray(cw2, dtype=_BF)
    b1tv = np.ascontiguousarray(
        np.asarray(b1, np.float32).reshape(H, 8, 128).transpose(2, 0, 1)
        .reshape(128, H * 8))
    b2tv = np.ascontiguousarray(
        np.asarray(b2, np.float32).reshape(H, 2, 128).transpose(2, 0, 1)
        .reshape(128, 8))
    cb1v = np.asarray(cb1, np.float32).reshape(SQ, 1)
    cb2tv = np.ascontiguousarray(
        np.asarray(cb2, np.float32).reshape(8, 128).T)

    shared = {
        "w1": w1b, "w2": w2b, "b1t": b1tv, "b2t": b2tv,
        "cw1": cw1b, "cb1t": cb1v, "cw2": cw2b, "cb2t": cb2tv,
    }
    return [dict(shared, xt=np.ascontiguousarray(xb[i].T))
            for i in range(NCORES)]


def kernel(x, W1, b1, W2, b2, cw1, cb1, cw2, cb2):
    from concourse.bass_utils import run_bass_kernel_spmd

    nc = _get_nc()
    in_maps = _make_in_maps(x, W1, b1, W2, b2, cw1, cb1, cw2, cb2)
    res = run_bass_kernel_spmd(nc, in_maps, core_ids=list(range(NCORES)))
    # un-transpose: per-core (DIM, N) -> (N, DIM)
    y = np.stack([res.results[i]["outT"].T for i in range(NCORES)], axis=0)
    return y.astype(np.float32)


# revision 17
# speedup vs baseline: 1.3514x; 1.3514x over previous
"""MultiHeadMlp TRN2 kernel: grouped per-head MLP + SE channel attention.

Full-input contract: kernel(**inputs) takes the complete arrays and returns
the complete output. Internally shards data-parallel over the batch dim
(B=8 -> 8 NeuronCores), builds one SPMD Bass/Tile program, and runs it via
run_bass_kernel_spmd.

Math (per batch element b, all tokens local to one core):
    xh = x.reshape(N, H, D)
    h  = gelu(xh @ W1 + b1)          per head, D=256 -> HID=1024
    o  = h @ W2 + b2                 per head, HID   -> D
    out = concat_heads(o)            (N, C)
    pooled = out.mean(axis=0)        (C,)
    gate = sigmoid(relu(pooled@cw1+cb1)@cw2+cb2)
    y = out * (1 + gate)

Layout strategy: everything on-chip is channel-major ("transposed"):
the host hands the kernel x^T (and un-transposes y^T on the way out), so
W1 [D,HID] / W2 [HID,D] serve directly as matmul lhsT operands, the SE
pool is a free-dim reduction, the gate is a native per-partition scalar
multiply, and the device never transposes anything.
"""

import numpy as np
import ml_dtypes

B = 8
N = 4096
DIM = 1024
H = 4
HD = 256           # head dim
HID = 1024         # per-head hidden
SQ = 64            # squeeze dim
TCH = 512          # tokens per chunk
NCHUNK = N // TCH  # 8
NCORES = 8

_BF = ml_dtypes.bfloat16

_cache = {}


def _build():
    from contextlib import ExitStack

    import concourse.bass as bass
    import concourse.mybir as mybir
    from concourse import bacc
    from concourse.tile import TileContext

    dt = mybir.dt
    bf = dt.bfloat16
    f32 = dt.float32
    Act = mybir.ActivationFunctionType
    Alu = mybir.AluOpType
    Ax = mybir.AxisListType

    nc = bacc.Bacc("TRN2", target_bir_lowering=False, debug=False)

    xt = nc.dram_tensor("xt", [DIM, N], bf, kind="ExternalInput")
    w1 = nc.dram_tensor("w1", [H, HD, HID], bf, kind="ExternalInput")
    w2 = nc.dram_tensor("w2", [H, HID, HD], bf, kind="ExternalInput")
    b1t = nc.dram_tensor("b1t", [128, H * 8], f32, kind="ExternalInput")
    b2t = nc.dram_tensor("b2t", [128, 8], f32, kind="ExternalInput")
    cw1 = nc.dram_tensor("cw1", [DIM, SQ], bf, kind="ExternalInput")
    cb1t = nc.dram_tensor("cb1t", [SQ, 1], f32, kind="ExternalInput")
    cw2 = nc.dram_tensor("cw2", [SQ, DIM], bf, kind="ExternalInput")
    cb2t = nc.dram_tensor("cb2t", [128, 8], f32, kind="ExternalInput")
    outT = nc.dram_tensor("outT", [DIM, N], bf, kind="ExternalOutput")

    with TileContext(nc) as tc, ExitStack() as ctx:
        const = ctx.enter_context(tc.tile_pool(name="const", bufs=1))
        hpool = ctx.enter_context(tc.tile_pool(name="hpool", bufs=2))
        pg1 = ctx.enter_context(tc.tile_pool(name="pg1", bufs=3, space="PSUM"))
        pg2 = ctx.enter_context(tc.tile_pool(name="pg2", bufs=3, space="PSUM"))

        # ---- activation-table warmup (overlaps the load phase) ----
        warm = const.tile([128, 1], f32, name="warm", tag="warm")
        nc.vector.memset(warm, 0.0)
        nc.scalar.activation(out=warm, in_=warm, func=Act.Sigmoid)
        nc.scalar.activation(out=warm, in_=warm, func=Act.Relu)
        nc.scalar.activation(out=warm, in_=warm, func=Act.Gelu)

        # ---- weights + x^T, ordered by first use, all on HWDGE ----
        # consolidated per-head weight tiles: one DMA each
        w1sb = [const.tile([128, 2, HID], bf, name=f"w1sb_{h}",
                           tag=f"w1sb_{h}") for h in range(H)]
        w2sb = [const.tile([128, 8, HD], bf, name=f"w2sb_{h}",
                           tag=f"w2sb_{h}") for h in range(H)]
        xfull = [const.tile([128, N], bf, name=f"xfull_{c}",
                            tag=f"xfull_{c}") for c in range(8)]
        b1sb = const.tile([128, H * 8], f32, name="b1sb", tag="b1sb")
        b2sb = const.tile([128, 8], f32, name="b2sb", tag="b2sb")

        HN = N // 2
        nc.sync.dma_start(out=w1sb[0],
                          in_=w1[0].rearrange("(k p) n -> p k n", p=128))
        nc.sync.dma_start(out=b1sb, in_=b1t[:, :])
        # first chunk's x slices first for the earliest possible PE start
        nc.sync.dma_start(out=xfull[0][:, :TCH], in_=xt[0:128, :TCH])
        nc.sync.dma_start(out=xfull[1][:, :TCH], in_=xt[128:256, :TCH])
        nc.sync.dma_start(out=xfull[0][:, TCH:HN], in_=xt[0:128, TCH:HN])
        nc.sync.dma_start(out=xfull[1][:, TCH:HN], in_=xt[128:256, TCH:HN])
        nc.sync.dma_start(out=w2sb[0],
                          in_=w2[0].rearrange("(k p) n -> p k n", p=128))
        nc.sync.dma_start(out=b2sb, in_=b2t[:, :])
        for h in range(1, H):
            nc.sync.dma_start(out=w1sb[h],
                              in_=w1[h].rearrange("(k p) n -> p k n", p=128))
            nc.sync.dma_start(out=xfull[2 * h][:, :HN],
                              in_=xt[h * 256:h * 256 + 128, :HN])
            nc.sync.dma_start(out=xfull[2 * h + 1][:, :HN],
                              in_=xt[h * 256 + 128:(h + 1) * 256, :HN])
            nc.sync.dma_start(out=w2sb[h],
                              in_=w2[h].rearrange("(k p) n -> p k n", p=128))
        for c in range(8):
            nc.sync.dma_start(out=xfull[c][:, HN:],
                              in_=xt[c * 128:(c + 1) * 128, HN:])
        cw1sb = const.tile([128, 8, SQ], bf, name="cw1sb", tag="cw1sb")
        nc.sync.dma_start(out=cw1sb,
                          in_=cw1.rearrange("(c p) n -> p c n", p=128))
        cb1sb = const.tile([SQ, 1], f32, name="cb1sb", tag="cb1sb")
        nc.sync.dma_start(out=cb1sb, in_=cb1t[:, :])
        cw2sb = const.tile([SQ, DIM], bf, name="cw2sb", tag="cw2sb")
        nc.sync.dma_start(out=cw2sb, in_=cw2[:, :])
        cb2sb = const.tile([128, 8], f32, name="cb2sb", tag="cb2sb")
        nc.sync.dma_start(out=cb2sb, in_=cb2t[:, :])

        # channel-major out accumulator (persists across whole kernel)
        oT = []
        for c in range(8):
            t = const.tile([128, N], bf, name=f"oT_{c}", tag=f"oT_{c}")
            oT.append(t)
        # per-(chunk, chan-tile) row sums for the SE pool
        prow = const.tile([128, NCHUNK * 8], f32, name="prow", tag="prow")

        # ---- main loop over token chunks ----
        for i in range(NCHUNK):
            t0 = i * TCH
            for h in range(H):
                # GEMM1: h^T[m-tile] = gelu(W1_h^T x^T + b1)
                ht = []
                for m in range(8):
                    p1 = pg1.tile([128, TCH], f32, name="p1", tag="p1")
                    nc.tensor.matmul(
                        p1, lhsT=w1sb[h][:, 0, m * 128:(m + 1) * 128],
                        rhs=xfull[2 * h][:, t0:t0 + TCH],
                        start=True, stop=False)
                    nc.tensor.matmul(
                        p1, lhsT=w1sb[h][:, 1, m * 128:(m + 1) * 128],
                        rhs=xfull[2 * h + 1][:, t0:t0 + TCH],
                        start=False, stop=True)
                    hm = hpool.tile([128, TCH], bf, name=f"ht_{m}",
                                    tag=f"ht_{m}")
                    nc.scalar.activation(
                        out=hm, in_=p1, func=Act.Gelu,
                        bias=b1sb[:, h * 8 + m:h * 8 + m + 1])
                    ht.append(hm)
                # GEMM2: o^T[d-half] = W2_h^T h^T + b2
                for d in range(2):
                    c = h * 2 + d
                    p2 = pg2.tile([128, TCH], f32, name="p2", tag="p2")
                    for k in range(8):
                        nc.tensor.matmul(
                            p2, lhsT=w2sb[h][:, k, d * 128:(d + 1) * 128],
                            rhs=ht[k], start=(k == 0), stop=(k == 7))
                    nc.vector.tensor_scalar(
                        out=oT[c][:, t0:t0 + TCH], in0=p2,
                        scalar1=b2sb[:, c:c + 1],
                        scalar2=0.0, op0=Alu.add, op1=Alu.add,
                        accum_out=prow[:, i * 8 + c:i * 8 + c + 1])

        # ---- SE channel attention on pooled means (all channel-major) ----
        # partial reduction over chunks 0..6 runs as soon as those chunks'
        # row sums exist (overlaps chunk 7 compute); only the final add is
        # on the critical path.
        pooled_part = const.tile([128, 8], f32, name="pooled_part",
                                 tag="pooled_part")
        pooled_raw = const.tile([128, 8], f32, name="pooled_raw",
                                tag="pooled_raw")
        prow3 = prow.rearrange("p (i c) -> p i c", c=8)
        for c in range(8):
            nc.vector.tensor_reduce(
                out=pooled_part[:, c:c + 1], in_=prow3[:, 0:NCHUNK - 1, c],
                axis=Ax.X, op=Alu.add)
        nc.vector.tensor_tensor(out=pooled_raw, in0=pooled_part,
                                in1=prow3[:, NCHUNK - 1, :], op=Alu.add)
        pooledT = const.tile([128, 8], bf, name="pooledT", tag="pooledT")
        nc.vector.tensor_scalar_mul(pooledT, pooled_raw, 1.0 / N)

        pz = pg1.tile([SQ, 1], f32, name="pz", tag="p1")
        for c in range(8):
            nc.tensor.matmul(pz, lhsT=cw1sb[:, c, :], rhs=pooledT[:, c:c + 1],
                             start=(c == 0), stop=(c == 7))
        z1sb = const.tile([SQ, 1], bf, name="z1sb", tag="z1sb")
        nc.scalar.activation(out=z1sb, in_=pz, func=Act.Relu, bias=cb1sb)

        # gate^T[c] = 1 + sigmoid(cw2^T z1 + cb2), per chan-tile
        g1T = const.tile([128, 8], f32, name="g1T", tag="g1T")
        for c in range(8):
            gp = pg2.tile([128, 1], f32, name="gp", tag="p2")
            nc.tensor.matmul(gp, lhsT=cw2sb[:, c * 128:(c + 1) * 128],
                             rhs=z1sb, start=True, stop=True)
            nc.scalar.activation(out=g1T[:, c:c + 1], in_=gp,
                                 func=Act.Sigmoid, bias=cb2sb[:, c:c + 1])
        nc.vector.tensor_scalar_add(g1T, g1T, 1.0)

        # ---- final scale + store (in-place on oT; DVE with GpSimd assist) ----
        for c in range(8):
            for half in range(2):
                sl = slice(half * 2048, (half + 1) * 2048)
                nc.vector.tensor_scalar_mul(
                    oT[c][:, sl], oT[c][:, sl], g1T[:, c:c + 1])
                nc.sync.dma_start(out=outT[c * 128:(c + 1) * 128, sl],
                                  in_=oT[c][:, sl])

    nc.compile()
    return nc


def _get_nc():
    if "nc" not in _cache:
        _cache["nc"] = _build()
    return _cache["nc"]


def _make_in_maps(x, W1, b1, W2, b2, cw1, cb1, cw2, cb2):
    # bf16 + pre-transposed x: (B, N, DIM) -> per-core (DIM, N)
    xb = np.asarray(x, dtype=_BF)
    w1b = np.asarray(W1, dtype=_BF)
    w2b = np.asarray(W2, dtype=_BF)
    cw1b = np.asarray(cw1, dtype=_BF)
    cw2b = np.asarray(cw2, dtype=_BF)
    b1tv = np.ascontiguousarray(
        np.asarray(b1, np.float32).reshape(H, 8, 128).transpose(2, 0, 1)
        .reshape(128, H * 8))
    b2tv = np.ascontiguousarray(
        np.asarray(b2, np.float32).reshape(H, 2, 128).transpose(2, 0, 1)
        .reshape(128, 8))
    cb1v = np.asarray(cb1, np.float32).reshape(SQ, 1)
    cb2tv = np.ascontiguousarray(
        np.asarray(cb2, np.float32).reshape(8, 128).T)

    shared = {
        "w1": w1b, "w2": w2b, "b1t": b1tv, "b2t": b2tv,
        "cw1": cw1b, "cb1t": cb1v, "cw2": cw2b, "cb2t": cb2tv,
    }
    return [dict(shared, xt=np.ascontiguousarray(xb[i].T))
            for i in range(NCORES)]


def kernel(x, W1, b1, W2, b2, cw1, cb1, cw2, cb2):
    from concourse.bass_utils import run_bass_kernel_spmd

    nc = _get_nc()
    in_maps = _make_in_maps(x, W1, b1, W2, b2, cw1, cb1, cw2, cb2)
    res = run_bass_kernel_spmd(nc, in_maps, core_ids=list(range(NCORES)))
    # un-transpose: per-core (DIM, N) -> (N, DIM)
    y = np.stack([res.results[i]["outT"].T for i in range(NCORES)], axis=0)
    return y.astype(np.float32)


# revision 20
# speedup vs baseline: 1.3849x; 1.0248x over previous
"""MultiHeadMlp TRN2 kernel: grouped per-head MLP + SE channel attention.

Full-input contract: kernel(**inputs) takes the complete arrays and returns
the complete output. Internally shards data-parallel over the batch dim
(B=8 -> 8 NeuronCores), builds one SPMD Bass/Tile program, and runs it via
run_bass_kernel_spmd.

Math (per batch element b, all tokens local to one core):
    xh = x.reshape(N, H, D)
    h  = gelu(xh @ W1 + b1)          per head, D=256 -> HID=1024
    o  = h @ W2 + b2                 per head, HID   -> D
    out = concat_heads(o)            (N, C)
    pooled = out.mean(axis=0)        (C,)
    gate = sigmoid(relu(pooled@cw1+cb1)@cw2+cb2)
    y = out * (1 + gate)

Layout strategy: everything on-chip is channel-major ("transposed"):
the host hands the kernel x^T (and un-transposes y^T on the way out), so
W1 [D,HID] / W2 [HID,D] serve directly as matmul lhsT operands, the SE
pool is a free-dim reduction, the gate is a native per-partition scalar
multiply, and the device never transposes anything.
"""

import numpy as np
import ml_dtypes

B = 8
N = 4096
DIM = 1024
H = 4
HD = 256           # head dim
HID = 1024         # per-head hidden
SQ = 64            # squeeze dim
TCH = 512          # tokens per chunk
NCHUNK = N // TCH  # 8
NCORES = 8

_BF = ml_dtypes.bfloat16

_cache = {}


def _build():
    from contextlib import ExitStack

    import concourse.bass as bass
    import concourse.mybir as mybir
    from concourse import bacc
    from concourse.tile import TileContext

    dt = mybir.dt
    bf = dt.bfloat16
    f32 = dt.float32
    Act = mybir.ActivationFunctionType
    Alu = mybir.AluOpType
    Ax = mybir.AxisListType

    nc = bacc.Bacc("TRN2", target_bir_lowering=False, debug=False)

    xt = nc.dram_tensor("xt", [DIM, N], bf, kind="ExternalInput")
    w1 = nc.dram_tensor("w1", [H, HD, HID], bf, kind="ExternalInput")
    w2 = nc.dram_tensor("w2", [H, HID, HD], bf, kind="ExternalInput")
    b1t = nc.dram_tensor("b1t", [128, H * 8], f32, kind="ExternalInput")
    b2t = nc.dram_tensor("b2t", [128, 8], f32, kind="ExternalInput")
    cw1 = nc.dram_tensor("cw1", [DIM, SQ], bf, kind="ExternalInput")
    cb1t = nc.dram_tensor("cb1t", [SQ, 1], f32, kind="ExternalInput")
    cw2 = nc.dram_tensor("cw2", [SQ, DIM], bf, kind="ExternalInput")
    cb2t = nc.dram_tensor("cb2t", [128, 8], f32, kind="ExternalInput")
    outT = nc.dram_tensor("outT", [DIM, N], bf, kind="ExternalOutput")

    with TileContext(nc) as tc, ExitStack() as ctx:
        const = ctx.enter_context(tc.tile_pool(name="const", bufs=1))
        hpool = ctx.enter_context(tc.tile_pool(name="hpool", bufs=2))
        pg1 = ctx.enter_context(tc.tile_pool(name="pg1", bufs=3, space="PSUM"))
        pg2 = ctx.enter_context(tc.tile_pool(name="pg2", bufs=3, space="PSUM"))

        # ---- activation-table + PE-clock warmup (overlaps the load phase) ----
        warm = const.tile([128, 1], f32, name="warm", tag="warm")
        nc.vector.memset(warm, 0.0)
        nc.scalar.activation(out=warm, in_=warm, func=Act.Sigmoid)
        nc.scalar.activation(out=warm, in_=warm, func=Act.Relu)
        nc.scalar.activation(out=warm, in_=warm, func=Act.Gelu)
        # dummy matmuls keep the PE busy through the HAM activity window so
        # the real GEMM stream starts at the warm 2.4 GHz clock
        wmm = const.tile([128, 512], bf, name="wmm", tag="wmm")
        nc.vector.memset(wmm, 0.0)
        for _ in range(40):
            pw = pg1.tile([128, 512], f32, name="p1", tag="p1")
            nc.tensor.matmul(pw, lhsT=wmm[:, 0:128], rhs=wmm,
                             start=True, stop=True)

        # ---- weights + x^T, ordered by first use, all on HWDGE ----
        # consolidated per-head weight tiles: one DMA each
        w1sb = [const.tile([128, 2, HID], bf, name=f"w1sb_{h}",
                           tag=f"w1sb_{h}") for h in range(H)]
        w2sb = [const.tile([128, 8, HD], bf, name=f"w2sb_{h}",
                           tag=f"w2sb_{h}") for h in range(H)]
        xfull = [const.tile([128, N], bf, name=f"xfull_{c}",
                            tag=f"xfull_{c}") for c in range(8)]
        b1sb = const.tile([128, H * 8], f32, name="b1sb", tag="b1sb")
        b2sb = const.tile([128, 8], f32, name="b2sb", tag="b2sb")

        HN = N // 2
        w1r0 = w1[0].rearrange("(k p) n -> p k n", p=128)
        nc.sync.dma_start(out=w1sb[0][:, 0:1, :], in_=w1r0[:, 0:1, :])
        nc.sync.dma_start(out=w1sb[0][:, 1:2, :], in_=w1r0[:, 1:2, :])
        nc.sync.dma_start(out=b1sb, in_=b1t[:, :])
        # first chunk's x slices first for the earliest possible PE start
        nc.sync.dma_start(out=xfull[0][:, :TCH], in_=xt[0:128, :TCH])
        nc.sync.dma_start(out=xfull[1][:, :TCH], in_=xt[128:256, :TCH])
        nc.sync.dma_start(out=xfull[0][:, TCH:HN], in_=xt[0:128, TCH:HN])
        nc.sync.dma_start(out=xfull[1][:, TCH:HN], in_=xt[128:256, TCH:HN])
        nc.sync.dma_start(out=w2sb[0],
                          in_=w2[0].rearrange("(k p) n -> p k n", p=128))
        nc.sync.dma_start(out=b2sb, in_=b2t[:, :])
        for h in range(1, H):
            nc.sync.dma_start(out=w1sb[h],
                              in_=w1[h].rearrange("(k p) n -> p k n", p=128))
            nc.sync.dma_start(out=xfull[2 * h][:, :HN],
                              in_=xt[h * 256:h * 256 + 128, :HN])
            nc.sync.dma_start(out=xfull[2 * h + 1][:, :HN],
                              in_=xt[h * 256 + 128:(h + 1) * 256, :HN])
            nc.sync.dma_start(out=w2sb[h],
                              in_=w2[h].rearrange("(k p) n -> p k n", p=128))
        for c in range(8):
            nc.sync.dma_start(out=xfull[c][:, HN:],
                              in_=xt[c * 128:(c + 1) * 128, HN:])
        cw1sb = const.tile([128, 8, SQ], bf, name="cw1sb", tag="cw1sb")
        nc.sync.dma_start(out=cw1sb,
                          in_=cw1.rearrange("(c p) n -> p c n", p=128))
        cb1sb = const.tile([SQ, 1], f32, name="cb1sb", tag="cb1sb")
        nc.sync.dma_start(out=cb1sb, in_=cb1t[:, :])
        cw2sb = const.tile([SQ, DIM], bf, name="cw2sb", tag="cw2sb")
        nc.sync.dma_start(out=cw2sb, in_=cw2[:, :])
        cb2sb = const.tile([128, 8], f32, name="cb2sb", tag="cb2sb")
        nc.sync.dma_start(out=cb2sb, in_=cb2t[:, :])

        # channel-major out accumulator (persists across whole kernel)
        oT = []
        for c in range(8):
            t = const.tile([128, N], bf, name=f"oT_{c}", tag=f"oT_{c}")
            oT.append(t)
        # per-(chunk, chan-tile) row sums for the SE pool
        prow = const.tile([128, NCHUNK * 8], f32, name="prow", tag="prow")

        # ---- main loop over token chunks ----
        for i in range(NCHUNK):
            t0 = i * TCH
            for h in range(H):
                # GEMM1: h^T[m-tile] = gelu(W1_h^T x^T + b1)
                ht = []
                for m in range(8):
                    p1 = pg1.tile([128, TCH], f32, name="p1", tag="p1")
                    nc.tensor.matmul(
                        p1, lhsT=w1sb[h][:, 0, m * 128:(m + 1) * 128],
                        rhs=xfull[2 * h][:, t0:t0 + TCH],
                        start=True, stop=False)
                    nc.tensor.matmul(
                        p1, lhsT=w1sb[h][:, 1, m * 128:(m + 1) * 128],
                        rhs=xfull[2 * h + 1][:, t0:t0 + TCH],
                        start=False, stop=True)
                    hm = hpool.tile([128, TCH], bf, name=f"ht_{m}",
                                    tag=f"ht_{m}")
                    nc.scalar.activation(
                        out=hm, in_=p1, func=Act.Gelu,
                        bias=b1sb[:, h * 8 + m:h * 8 + m + 1])
                    ht.append(hm)
                # GEMM2: o^T[d-half] = W2_h^T h^T + b2
                for d in range(2):
                    c = h * 2 + d
                    p2 = pg2.tile([128, TCH], f32, name="p2", tag="p2")
                    for k in range(8):
                        nc.tensor.matmul(
                            p2, lhsT=w2sb[h][:, k, d * 128:(d + 1) * 128],
                            rhs=ht[k], start=(k == 0), stop=(k == 7))
                    nc.vector.tensor_scalar(
                        out=oT[c][:, t0:t0 + TCH], in0=p2,
                        scalar1=b2sb[:, c:c + 1],
                        scalar2=0.0, op0=Alu.add, op1=Alu.add,
                        accum_out=prow[:, i * 8 + c:i * 8 + c + 1])

        # ---- SE channel attention on pooled means (all channel-major) ----
        # partial reduction over chunks 0..6 runs as soon as those chunks'
        # row sums exist (overlaps chunk 7 compute); only the final add is
        # on the critical path.
        pooled_part = const.tile([128, 8], f32, name="pooled_part",
                                 tag="pooled_part")
        pooled_raw = const.tile([128, 8], f32, name="pooled_raw",
                                tag="pooled_raw")
        prow3 = prow.rearrange("p (i c) -> p i c", c=8)
        for c in range(8):
            nc.vector.tensor_reduce(
                out=pooled_part[:, c:c + 1], in_=prow3[:, 0:NCHUNK - 1, c],
                axis=Ax.X, op=Alu.add)
        nc.vector.tensor_tensor(out=pooled_raw, in0=pooled_part,
                                in1=prow3[:, NCHUNK - 1, :], op=Alu.add)
        pooledT = const.tile([128, 8], bf, name="pooledT", tag="pooledT")
        nc.vector.tensor_scalar_mul(pooledT, pooled_raw, 1.0 / N)

        pz = pg1.tile([SQ, 1], f32, name="pz", tag="p1")
        for c in range(8):
            nc.tensor.matmul(pz, lhsT=cw1sb[:, c, :], rhs=pooledT[:, c:c + 1],
                             start=(c == 0), stop=(c == 7))
        z1sb = const.tile([SQ, 1], bf, name="z1sb", tag="z1sb")
        nc.scalar.activation(out=z1sb, in_=pz, func=Act.Relu, bias=cb1sb)

        # gate^T[c] = 1 + sigmoid(cw2^T z1 + cb2), per chan-tile
        g1T = const.tile([128, 8], f32, name="g1T", tag="g1T")
        for c in range(8):
            gp = pg2.tile([128, 1], f32, name="gp", tag="p2")
            nc.tensor.matmul(gp, lhsT=cw2sb[:, c * 128:(c + 1) * 128],
                             rhs=z1sb, start=True, stop=True)
            nc.scalar.activation(out=g1T[:, c:c + 1], in_=gp,
                                 func=Act.Sigmoid, bias=cb2sb[:, c:c + 1])
        nc.vector.tensor_scalar_add(g1T, g1T, 1.0)

        # ---- final scale + store (in-place on oT; DVE with GpSimd assist) ----
        for c in range(8):
            for half in range(2):
                sl = slice(half * 2048, (half + 1) * 2048)
                if (c, half) in ((0, 0), (2, 0), (4, 0), (6, 0)):
                    # ACT takes a few slices in parallel with the DVE stream
                    nc.scalar.activation(
                        out=oT[c][:, sl], in_=oT[c][:, sl],
                        func=Act.Copy, scale=g1T[:, c:c + 1])
                else:
                    nc.vector.tensor_scalar_mul(
                        oT[c][:, sl], oT[c][:, sl], g1T[:, c:c + 1])
                nc.sync.dma_start(out=outT[c * 128:(c + 1) * 128, sl],
                                  in_=oT[c][:, sl])

    nc.compile()
    return nc


def _get_nc():
    if "nc" not in _cache:
        _cache["nc"] = _build()
    return _cache["nc"]


def _make_in_maps(x, W1, b1, W2, b2, cw1, cb1, cw2, cb2):
    # bf16 + pre-transposed x: (B, N, DIM) -> per-core (DIM, N)
    xb = np.asarray(x, dtype=_BF)
    w1b = np.asarray(W1, dtype=_BF)
    w2b = np.asarray(W2, dtype=_BF)
    cw1b = np.asarray(cw1, dtype=_BF)
    cw2b = np.asarray(cw2, dtype=_BF)
    b1tv = np.ascontiguousarray(
        np.asarray(b1, np.float32).reshape(H, 8, 128).transpose(2, 0, 1)
        .reshape(128, H * 8))
    b2tv = np.ascontiguousarray(
        np.asarray(b2, np.float32).reshape(H, 2, 128).transpose(2, 0, 1)
        .reshape(128, 8))
    cb1v = np.asarray(cb1, np.float32).reshape(SQ, 1)
    cb2tv = np.ascontiguousarray(
        np.asarray(cb2, np.float32).reshape(8, 128).T)

    shared = {
        "w1": w1b, "w2": w2b, "b1t": b1tv, "b2t": b2tv,
        "cw1": cw1b, "cb1t": cb1v, "cw2": cw2b, "cb2t": cb2tv,
    }
    return [dict(shared, xt=np.ascontiguousarray(xb[i].T))
            for i in range(NCORES)]


def kernel(x, W1, b1, W2, b2, cw1, cb1, cw2, cb2):
    from concourse.bass_utils import run_bass_kernel_spmd

    nc = _get_nc()
    in_maps = _make_in_maps(x, W1, b1, W2, b2, cw1, cb1, cw2, cb2)
    res = run_bass_kernel_spmd(nc, in_maps, core_ids=list(range(NCORES)))
    # un-transpose: per-core (DIM, N) -> (N, DIM)
    y = np.stack([res.results[i]["outT"].T for i in range(NCORES)], axis=0)
    return y.astype(np.float32)


# revision 21
# speedup vs baseline: 1.4016x; 1.0121x over previous
"""MultiHeadMlp TRN2 kernel: grouped per-head MLP + SE channel attention.

Full-input contract: kernel(**inputs) takes the complete arrays and returns
the complete output. Internally shards data-parallel over the batch dim
(B=8 -> 8 NeuronCores), builds one SPMD Bass/Tile program, and runs it via
run_bass_kernel_spmd.

Math (per batch element b, all tokens local to one core):
    xh = x.reshape(N, H, D)
    h  = gelu(xh @ W1 + b1)          per head, D=256 -> HID=1024
    o  = h @ W2 + b2                 per head, HID   -> D
    out = concat_heads(o)            (N, C)
    pooled = out.mean(axis=0)        (C,)
    gate = sigmoid(relu(pooled@cw1+cb1)@cw2+cb2)
    y = out * (1 + gate)

Layout strategy: everything on-chip is channel-major ("transposed"):
the host hands the kernel x^T (and un-transposes y^T on the way out), so
W1 [D,HID] / W2 [HID,D] serve directly as matmul lhsT operands, the SE
pool is a free-dim reduction, the gate is a native per-partition scalar
multiply, and the device never transposes anything.
"""

import numpy as np
import ml_dtypes

B = 8
N = 4096
DIM = 1024
H = 4
HD = 256           # head dim
HID = 1024         # per-head hidden
SQ = 64            # squeeze dim
TCH = 512          # tokens per chunk
NCHUNK = N // TCH  # 8
NCORES = 8

_BF = ml_dtypes.bfloat16

_cache = {}


def _build():
    from contextlib import ExitStack

    import concourse.bass as bass
    import concourse.mybir as mybir
    from concourse import bacc
    from concourse.tile import TileContext

    dt = mybir.dt
    bf = dt.bfloat16
    f32 = dt.float32
    Act = mybir.ActivationFunctionType
    Alu = mybir.AluOpType
    Ax = mybir.AxisListType

    nc = bacc.Bacc("TRN2", target_bir_lowering=False, debug=False)

    xt = nc.dram_tensor("xt", [DIM, N], bf, kind="ExternalInput")
    w1 = nc.dram_tensor("w1", [H, HD, HID], bf, kind="ExternalInput")
    w2 = nc.dram_tensor("w2", [H, HID, HD], bf, kind="ExternalInput")
    b1t = nc.dram_tensor("b1t", [128, H * 8], f32, kind="ExternalInput")
    b2t = nc.dram_tensor("b2t", [128, 8], f32, kind="ExternalInput")
    cw1 = nc.dram_tensor("cw1", [DIM, SQ], bf, kind="ExternalInput")
    cb1t = nc.dram_tensor("cb1t", [SQ, 1], f32, kind="ExternalInput")
    cw2 = nc.dram_tensor("cw2", [SQ, DIM], bf, kind="ExternalInput")
    cb2t = nc.dram_tensor("cb2t", [128, 8], f32, kind="ExternalInput")
    outT = nc.dram_tensor("outT", [DIM, N], bf, kind="ExternalOutput")

    with TileContext(nc) as tc, ExitStack() as ctx:
        const = ctx.enter_context(tc.tile_pool(name="const", bufs=1))
        hpool = ctx.enter_context(tc.tile_pool(name="hpool", bufs=2))
        pg1 = ctx.enter_context(tc.tile_pool(name="pg1", bufs=3, space="PSUM"))
        pg2 = ctx.enter_context(tc.tile_pool(name="pg2", bufs=3, space="PSUM"))

        # ---- activation-table + PE-clock warmup (overlaps the load phase) ----
        warm = const.tile([128, 1], f32, name="warm", tag="warm")
        nc.vector.memset(warm, 0.0)
        nc.scalar.activation(out=warm, in_=warm, func=Act.Sigmoid)
        nc.scalar.activation(out=warm, in_=warm, func=Act.Relu)
        nc.scalar.activation(out=warm, in_=warm, func=Act.Gelu)
        # dummy matmuls keep the PE busy through the HAM activity window so
        # the real GEMM stream starts at the warm 2.4 GHz clock
        wmm = const.tile([128, 512], bf, name="wmm", tag="wmm")
        nc.vector.memset(wmm, 0.0)
        for _ in range(12):
            pw = pg1.tile([128, 512], f32, name="p1", tag="p1")
            nc.tensor.matmul(pw, lhsT=wmm[:, 0:128], rhs=wmm,
                             start=True, stop=True)

        # ---- weights + x^T, ordered by first use, all on HWDGE ----
        # consolidated per-head weight tiles: one DMA each
        w1sb = [const.tile([128, 2, HID], bf, name=f"w1sb_{h}",
                           tag=f"w1sb_{h}") for h in range(H)]
        w2sb = [const.tile([128, 8, HD], bf, name=f"w2sb_{h}",
                           tag=f"w2sb_{h}") for h in range(H)]
        xfull = [const.tile([128, N], bf, name=f"xfull_{c}",
                            tag=f"xfull_{c}") for c in range(8)]
        b1sb = const.tile([128, H * 8], f32, name="b1sb", tag="b1sb")
        b2sb = const.tile([128, 8], f32, name="b2sb", tag="b2sb")

        HN = N // 2
        w1r0 = w1[0].rearrange("(k p) n -> p k n", p=128)
        nc.sync.dma_start(out=w1sb[0][:, 0:1, :], in_=w1r0[:, 0:1, :])
        nc.sync.dma_start(out=w1sb[0][:, 1:2, :], in_=w1r0[:, 1:2, :])
        nc.sync.dma_start(out=b1sb, in_=b1t[:, :])
        # first chunk's x slices first for the earliest possible PE start
        nc.sync.dma_start(out=xfull[0][:, :TCH], in_=xt[0:128, :TCH])
        nc.sync.dma_start(out=xfull[1][:, :TCH], in_=xt[128:256, :TCH])
        nc.sync.dma_start(out=xfull[0][:, TCH:HN], in_=xt[0:128, TCH:HN])
        nc.sync.dma_start(out=xfull[1][:, TCH:HN], in_=xt[128:256, TCH:HN])
        nc.sync.dma_start(out=w2sb[0],
                          in_=w2[0].rearrange("(k p) n -> p k n", p=128))
        nc.sync.dma_start(out=b2sb, in_=b2t[:, :])
        for h in range(1, H):
            nc.sync.dma_start(out=w1sb[h],
                              in_=w1[h].rearrange("(k p) n -> p k n", p=128))
            nc.sync.dma_start(out=xfull[2 * h][:, :HN],
                              in_=xt[h * 256:h * 256 + 128, :HN])
            nc.sync.dma_start(out=xfull[2 * h + 1][:, :HN],
                              in_=xt[h * 256 + 128:(h + 1) * 256, :HN])
            nc.sync.dma_start(out=w2sb[h],
                              in_=w2[h].rearrange("(k p) n -> p k n", p=128))
        for c in range(8):
            nc.sync.dma_start(out=xfull[c][:, HN:],
                              in_=xt[c * 128:(c + 1) * 128, HN:])
        cw1sb = const.tile([128, 8, SQ], bf, name="cw1sb", tag="cw1sb")
        nc.sync.dma_start(out=cw1sb,
                          in_=cw1.rearrange("(c p) n -> p c n", p=128))
        cb1sb = const.tile([SQ, 1], f32, name="cb1sb", tag="cb1sb")
        nc.sync.dma_start(out=cb1sb, in_=cb1t[:, :])
        cw2sb = const.tile([SQ, DIM], bf, name="cw2sb", tag="cw2sb")
        nc.sync.dma_start(out=cw2sb, in_=cw2[:, :])
        cb2sb = const.tile([128, 8], f32, name="cb2sb", tag="cb2sb")
        nc.sync.dma_start(out=cb2sb, in_=cb2t[:, :])

        # channel-major out accumulator (persists across whole kernel)
        oT = []
        for c in range(8):
            t = const.tile([128, N], bf, name=f"oT_{c}", tag=f"oT_{c}")
            oT.append(t)
        # per-(chunk, chan-tile) row sums for the SE pool
        prow = const.tile([128, NCHUNK * 8], f32, name="prow", tag="prow")

        # ---- main loop over token chunks ----
        for i in range(NCHUNK):
            t0 = i * TCH
            for h in range(H):
                # GEMM1: h^T[m-tile] = gelu(W1_h^T x^T + b1)
                ht = []
                for m in range(8):
                    p1 = pg1.tile([128, TCH], f32, name="p1", tag="p1")
                    nc.tensor.matmul(
                        p1, lhsT=w1sb[h][:, 0, m * 128:(m + 1) * 128],
                        rhs=xfull[2 * h][:, t0:t0 + TCH],
                        start=True, stop=False)
                    nc.tensor.matmul(
                        p1, lhsT=w1sb[h][:, 1, m * 128:(m + 1) * 128],
                        rhs=xfull[2 * h + 1][:, t0:t0 + TCH],
                        start=False, stop=True)
                    hm = hpool.tile([128, TCH], bf, name=f"ht_{m}",
                                    tag=f"ht_{m}")
                    nc.scalar.activation(
                        out=hm, in_=p1, func=Act.Gelu,
                        bias=b1sb[:, h * 8 + m:h * 8 + m + 1])
                    ht.append(hm)
                # GEMM2: o^T[d-half] = W2_h^T h^T + b2
                for d in range(2):
                    c = h * 2 + d
                    p2 = pg2.tile([128, TCH], f32, name="p2", tag="p2")
                    for k in range(8):
                        nc.tensor.matmul(
                            p2, lhsT=w2sb[h][:, k, d * 128:(d + 1) * 128],
                            rhs=ht[k], start=(k == 0), stop=(k == 7))
                    nc.vector.tensor_scalar(
                        out=oT[c][:, t0:t0 + TCH], in0=p2,
                        scalar1=b2sb[:, c:c + 1],
                        scalar2=0.0, op0=Alu.add, op1=Alu.add,
                        accum_out=prow[:, i * 8 + c:i * 8 + c + 1])

        # ---- SE channel attention on pooled means (all channel-major) ----
        # partial reduction over chunks 0..6 runs as soon as those chunks'
        # row sums exist (overlaps chunk 7 compute); only the final add is
        # on the critical path.
        pooled_part = const.tile([128, 8], f32, name="pooled_part",
                                 tag="pooled_part")
        pooled_raw = const.tile([128, 8], f32, name="pooled_raw",
                                tag="pooled_raw")
        prow3 = prow.rearrange("p (i c) -> p i c", c=8)
        for c in range(8):
            nc.vector.tensor_reduce(
                out=pooled_part[:, c:c + 1], in_=prow3[:, 0:NCHUNK - 1, c],
                axis=Ax.X, op=Alu.add)
        nc.vector.tensor_tensor(out=pooled_raw, in0=pooled_part,
                                in1=prow3[:, NCHUNK - 1, :], op=Alu.add)
        pooledT = const.tile([128, 8], bf, name="pooledT", tag="pooledT")
        nc.vector.tensor_scalar_mul(pooledT, pooled_raw, 1.0 / N)

        pz = pg1.tile([SQ, 1], f32, name="pz", tag="p1")
        for c in range(8):
            nc.tensor.matmul(pz, lhsT=cw1sb[:, c, :], rhs=pooledT[:, c:c + 1],
                             start=(c == 0), stop=(c == 7))
        z1sb = const.tile([SQ, 1], bf, name="z1sb", tag="z1sb")
        nc.scalar.activation(out=z1sb, in_=pz, func=Act.Relu, bias=cb1sb)

        # gate^T[c] = 1 + sigmoid(cw2^T z1 + cb2), per chan-tile
        g1T = const.tile([128, 8], f32, name="g1T", tag="g1T")
        for c in range(8):
            gp = pg2.tile([128, 1], f32, name="gp", tag="p2")
            nc.tensor.matmul(gp, lhsT=cw2sb[:, c * 128:(c + 1) * 128],
                             rhs=z1sb, start=True, stop=True)
            nc.scalar.activation(out=g1T[:, c:c + 1], in_=gp,
                                 func=Act.Sigmoid, bias=cb2sb[:, c:c + 1])
        nc.vector.tensor_scalar_add(g1T, g1T, 1.0)

        # ---- final scale + store (in-place on oT; DVE with GpSimd assist) ----
        for c in range(8):
            for half in range(2):
                sl = slice(half * 2048, (half + 1) * 2048)
                if (c, half) in ((0, 0), (2, 0), (4, 0), (6, 0)):
                    # ACT takes a few slices in parallel with the DVE stream
                    nc.scalar.activation(
                        out=oT[c][:, sl], in_=oT[c][:, sl],
                        func=Act.Copy, scale=g1T[:, c:c + 1])
                else:
                    nc.vector.tensor_scalar_mul(
                        oT[c][:, sl], oT[c][:, sl], g1T[:, c:c + 1])
                nc.sync.dma_start(out=outT[c * 128:(c + 1) * 128, sl],
                                  in_=oT[c][:, sl])

    nc.compile()
    return nc


def _get_nc():
    if "nc" not in _cache:
        _cache["nc"] = _build()
    return _cache["nc"]


def _make_in_maps(x, W1, b1, W2, b2, cw1, cb1, cw2, cb2):
    # bf16 + pre-transposed x: (B, N, DIM) -> per-core (DIM, N)
    xb = np.asarray(x, dtype=_BF)
    w1b = np.asarray(W1, dtype=_BF)
    w2b = np.asarray(W2, dtype=_BF)
    cw1b = np.asarray(cw1, dtype=_BF)
    cw2b = np.asarray(cw2, dtype=_BF)
    b1tv = np.ascontiguousarray(
        np.asarray(b1, np.float32).reshape(H, 8, 128).transpose(2, 0, 1)
        .reshape(128, H * 8))
    b2tv = np.ascontiguousarray(
        np.asarray(b2, np.float32).reshape(H, 2, 128).transpose(2, 0, 1)
        .reshape(128, 8))
    cb1v = np.asarray(cb1, np.float32).reshape(SQ, 1)
    cb2tv = np.ascontiguousarray(
        np.asarray(cb2, np.float32).reshape(8, 128).T)

    shared = {
        "w1": w1b, "w2": w2b, "b1t": b1tv, "b2t": b2tv,
        "cw1": cw1b, "cb1t": cb1v, "cw2": cw2b, "cb2t": cb2tv,
    }
    return [dict(shared, xt=np.ascontiguousarray(xb[i].T))
            for i in range(NCORES)]


def kernel(x, W1, b1, W2, b2, cw1, cb1, cw2, cb2):
    from concourse.bass_utils import run_bass_kernel_spmd

    nc = _get_nc()
    in_maps = _make_in_maps(x, W1, b1, W2, b2, cw1, cb1, cw2, cb2)
    res = run_bass_kernel_spmd(nc, in_maps, core_ids=list(range(NCORES)))
    # un-transpose: per-core (DIM, N) -> (N, DIM)
    y = np.stack([res.results[i]["outT"].T for i in range(NCORES)], axis=0)
    return y.astype(np.float32)


# revision 24
# speedup vs baseline: 1.4091x; 1.0054x over previous
"""MultiHeadMlp TRN2 kernel: grouped per-head MLP + SE channel attention.

Full-input contract: kernel(**inputs) takes the complete arrays and returns
the complete output. Internally shards data-parallel over the batch dim
(B=8 -> 8 NeuronCores), builds one SPMD Bass/Tile program, and runs it via
run_bass_kernel_spmd.

Math (per batch element b, all tokens local to one core):
    xh = x.reshape(N, H, D)
    h  = gelu(xh @ W1 + b1)          per head, D=256 -> HID=1024
    o  = h @ W2 + b2                 per head, HID   -> D
    out = concat_heads(o)            (N, C)
    pooled = out.mean(axis=0)        (C,)
    gate = sigmoid(relu(pooled@cw1+cb1)@cw2+cb2)
    y = out * (1 + gate)

Layout strategy: everything on-chip is channel-major ("transposed"):
the host hands the kernel x^T (and un-transposes y^T on the way out), so
W1 [D,HID] / W2 [HID,D] serve directly as matmul lhsT operands, the SE
pool is a free-dim reduction, the gate is a native per-partition scalar
multiply, and the device never transposes anything.
"""

import numpy as np
import ml_dtypes

B = 8
N = 4096
DIM = 1024
H = 4
HD = 256           # head dim
HID = 1024         # per-head hidden
SQ = 64            # squeeze dim
TCH = 512          # tokens per chunk
NCHUNK = N // TCH  # 8
NCORES = 8

_BF = ml_dtypes.bfloat16

_cache = {}


def _build():
    from contextlib import ExitStack

    import concourse.bass as bass
    import concourse.mybir as mybir
    from concourse import bacc
    from concourse.tile import TileContext

    dt = mybir.dt
    bf = dt.bfloat16
    f32 = dt.float32
    Act = mybir.ActivationFunctionType
    Alu = mybir.AluOpType
    Ax = mybir.AxisListType

    nc = bacc.Bacc("TRN2", target_bir_lowering=False, debug=False)

    xt = nc.dram_tensor("xt", [DIM, N], bf, kind="ExternalInput")
    w1 = nc.dram_tensor("w1", [H, HD, HID], bf, kind="ExternalInput")
    w2 = nc.dram_tensor("w2", [H, HID, HD], bf, kind="ExternalInput")
    b1t = nc.dram_tensor("b1t", [128, H * 8], f32, kind="ExternalInput")
    b2t = nc.dram_tensor("b2t", [128, 8], f32, kind="ExternalInput")
    cw1 = nc.dram_tensor("cw1", [DIM, SQ], bf, kind="ExternalInput")
    cb1t = nc.dram_tensor("cb1t", [SQ, 1], f32, kind="ExternalInput")
    cw2 = nc.dram_tensor("cw2", [SQ, DIM], bf, kind="ExternalInput")
    cb2t = nc.dram_tensor("cb2t", [128, 8], f32, kind="ExternalInput")
    outT = nc.dram_tensor("outT", [DIM, N], bf, kind="ExternalOutput")

    with TileContext(nc) as tc, ExitStack() as ctx:
        const = ctx.enter_context(tc.tile_pool(name="const", bufs=1))
        hpool = ctx.enter_context(tc.tile_pool(name="hpool", bufs=2))
        pg1 = ctx.enter_context(tc.tile_pool(name="pg1", bufs=3, space="PSUM"))
        pg2 = ctx.enter_context(tc.tile_pool(name="pg2", bufs=3, space="PSUM"))

        # ---- activation-table + PE-clock warmup (overlaps the load phase) ----
        warm = const.tile([128, 1], f32, name="warm", tag="warm")
        nc.vector.memset(warm, 0.0)
        nc.scalar.activation(out=warm, in_=warm, func=Act.Sigmoid)
        nc.scalar.activation(out=warm, in_=warm, func=Act.Relu)
        nc.scalar.activation(out=warm, in_=warm, func=Act.Gelu)
        # dummy matmuls keep the PE busy through the HAM activity window so
        # the real GEMM stream starts at the warm 2.4 GHz clock
        wmm = const.tile([128, 512], bf, name="wmm", tag="wmm")
        nc.vector.memset(wmm, 0.0)
        for _ in range(12):
            pw = pg1.tile([128, 512], f32, name="p1", tag="p1")
            nc.tensor.matmul(pw, lhsT=wmm[:, 0:128], rhs=wmm,
                             start=True, stop=True)

        # ---- weights + x^T, ordered by first use, all on HWDGE ----
        # consolidated per-head weight tiles: one DMA each
        w1sb = [const.tile([128, 2, HID], bf, name=f"w1sb_{h}",
                           tag=f"w1sb_{h}") for h in range(H)]
        w2sb = [const.tile([128, 8, HD], bf, name=f"w2sb_{h}",
                           tag=f"w2sb_{h}") for h in range(H)]
        xfull = [const.tile([128, N], bf, name=f"xfull_{c}",
                            tag=f"xfull_{c}") for c in range(8)]
        b1sb = const.tile([128, H * 8], f32, name="b1sb", tag="b1sb")
        b2sb = const.tile([128, 8], f32, name="b2sb", tag="b2sb")

        HN = N // 2
        w1r0 = w1[0].rearrange("(k p) n -> p k n", p=128)
        nc.sync.dma_start(out=w1sb[0][:, 0:1, :], in_=w1r0[:, 0:1, :])
        nc.sync.dma_start(out=w1sb[0][:, 1:2, :], in_=w1r0[:, 1:2, :])
        nc.sync.dma_start(out=b1sb, in_=b1t[:, :])
        # first chunk's x slices first for the earliest possible PE start
        nc.sync.dma_start(out=xfull[0][:, :TCH], in_=xt[0:128, :TCH])
        nc.sync.dma_start(out=xfull[1][:, :TCH], in_=xt[128:256, :TCH])
        nc.sync.dma_start(out=xfull[0][:, TCH:HN], in_=xt[0:128, TCH:HN])
        nc.sync.dma_start(out=xfull[1][:, TCH:HN], in_=xt[128:256, TCH:HN])
        nc.sync.dma_start(out=w2sb[0],
                          in_=w2[0].rearrange("(k p) n -> p k n", p=128))
        nc.sync.dma_start(out=b2sb, in_=b2t[:, :])
        for h in range(1, H):
            nc.sync.dma_start(out=w1sb[h],
                              in_=w1[h].rearrange("(k p) n -> p k n", p=128))
            nc.sync.dma_start(out=xfull[2 * h][:, :HN],
                              in_=xt[h * 256:h * 256 + 128, :HN])
            nc.sync.dma_start(out=xfull[2 * h + 1][:, :HN],
                              in_=xt[h * 256 + 128:(h + 1) * 256, :HN])
            nc.sync.dma_start(out=w2sb[h],
                              in_=w2[h].rearrange("(k p) n -> p k n", p=128))
        for c in range(8):
            nc.sync.dma_start(out=xfull[c][:, HN:],
                              in_=xt[c * 128:(c + 1) * 128, HN:])
        cw1sb = const.tile([128, 8, SQ], bf, name="cw1sb", tag="cw1sb")
        nc.sync.dma_start(out=cw1sb,
                          in_=cw1.rearrange("(c p) n -> p c n", p=128))
        cb1sb = const.tile([SQ, 1], f32, name="cb1sb", tag="cb1sb")
        nc.sync.dma_start(out=cb1sb, in_=cb1t[:, :])
        cw2sb = const.tile([SQ, DIM], bf, name="cw2sb", tag="cw2sb")
        nc.sync.dma_start(out=cw2sb, in_=cw2[:, :])
        cb2sb = const.tile([128, 8], f32, name="cb2sb", tag="cb2sb")
        nc.sync.dma_start(out=cb2sb, in_=cb2t[:, :])

        # channel-major out accumulator (persists across whole kernel)
        oT = []
        for c in range(8):
            t = const.tile([128, N], bf, name=f"oT_{c}", tag=f"oT_{c}")
            oT.append(t)
        # per-(chunk, chan-tile) row sums for the SE pool
        prow = const.tile([128, NCHUNK * 8], f32, name="prow", tag="prow")

        # ---- main loop over token chunks ----
        for i in range(NCHUNK):
            t0 = i * TCH
            for h in range(H):
                # GEMM1: h^T[m-tile] = gelu(W1_h^T x^T + b1)
                ht = []
                for m in range(8):
                    p1 = pg1.tile([128, TCH], f32, name="p1", tag="p1")
                    nc.tensor.matmul(
                        p1, lhsT=w1sb[h][:, 0, m * 128:(m + 1) * 128],
                        rhs=xfull[2 * h][:, t0:t0 + TCH],
                        start=True, stop=False)
                    nc.tensor.matmul(
                        p1, lhsT=w1sb[h][:, 1, m * 128:(m + 1) * 128],
                        rhs=xfull[2 * h + 1][:, t0:t0 + TCH],
                        start=False, stop=True)
                    hm = hpool.tile([128, TCH], bf, name=f"ht_{m}",
                                    tag=f"ht_{m}")
                    nc.scalar.activation(
                        out=hm, in_=p1, func=Act.Gelu,
                        bias=b1sb[:, h * 8 + m:h * 8 + m + 1])
                    ht.append(hm)
                # GEMM2: o^T[d-half] = W2_h^T h^T + b2
                for d in range(2):
                    c = h * 2 + d
                    p2 = pg2.tile([128, TCH], f32, name="p2", tag="p2")
                    for k in range(8):
                        nc.tensor.matmul(
                            p2, lhsT=w2sb[h][:, k, d * 128:(d + 1) * 128],
                            rhs=ht[k], start=(k == 0), stop=(k == 7))
                    nc.vector.tensor_scalar(
                        out=oT[c][:, t0:t0 + TCH], in0=p2,
                        scalar1=b2sb[:, c:c + 1],
                        scalar2=0.0, op0=Alu.add, op1=Alu.add,
                        accum_out=prow[:, i * 8 + c:i * 8 + c + 1])

        # prefetch the sigmoid table set while the SE reduction chain runs
        # (the main loop's gelus keep the gelu set resident; without this the
        # 1.3us table load lands between relu and the gate sigmoid)
        nc.scalar.activation(out=warm, in_=warm, func=Act.Sigmoid)

        # ---- SE channel attention on pooled means (all channel-major) ----
        # partial reduction over chunks 0..6 runs as soon as those chunks'
        # row sums exist (overlaps chunk 7 compute); only the final add is
        # on the critical path.
        pooled_part = const.tile([128, 8], f32, name="pooled_part",
                                 tag="pooled_part")
        pooled_raw = const.tile([128, 8], f32, name="pooled_raw",
                                tag="pooled_raw")
        prow3 = prow.rearrange("p (i c) -> p i c", c=8)
        for c in range(8):
            nc.vector.tensor_reduce(
                out=pooled_part[:, c:c + 1], in_=prow3[:, 0:NCHUNK - 1, c],
                axis=Ax.X, op=Alu.add)
        nc.vector.tensor_tensor(out=pooled_raw, in0=pooled_part,
                                in1=prow3[:, NCHUNK - 1, :], op=Alu.add)
        pooledT = const.tile([128, 8], bf, name="pooledT", tag="pooledT")
        nc.vector.tensor_scalar_mul(pooledT, pooled_raw, 1.0 / N)

        pz = pg1.tile([SQ, 1], f32, name="pz", tag="p1")
        for c in range(8):
            nc.tensor.matmul(pz, lhsT=cw1sb[:, c, :], rhs=pooledT[:, c:c + 1],
                             start=(c == 0), stop=(c == 7))
        z1sb = const.tile([SQ, 1], bf, name="z1sb", tag="z1sb")
        nc.scalar.activation(out=z1sb, in_=pz, func=Act.Relu, bias=cb1sb)

        # gate^T = 1 + sigmoid(cw2^T z1 + cb2): one psum tile, one sigmoid
        g1T = const.tile([128, 8], f32, name="g1T", tag="g1T")
        gp8 = pg2.tile([128, 8], f32, name="gp8", tag="p2")
        for c in range(8):
            nc.tensor.matmul(gp8[:, c:c + 1],
                             lhsT=cw2sb[:, c * 128:(c + 1) * 128],
                             rhs=z1sb, start=True, stop=True)
        gadd = const.tile([128, 8], f32, name="gadd", tag="gadd")
        nc.vector.tensor_tensor(out=gadd, in0=gp8, in1=cb2sb, op=Alu.add)
        nc.scalar.activation(out=g1T, in_=gadd, func=Act.Sigmoid)
        nc.vector.tensor_scalar_add(g1T, g1T, 1.0)

        # ---- final scale + store (in-place on oT; DVE with GpSimd assist) ----
        for c in range(8):
            for half in range(2):
                sl = slice(half * 2048, (half + 1) * 2048)
                if (c, half) in ((4, 1), (5, 1), (6, 1), (7, 1)):
                    # ACT takes a few slices in parallel with the DVE stream
                    nc.scalar.activation(
                        out=oT[c][:, sl], in_=oT[c][:, sl],
                        func=Act.Copy, scale=g1T[:, c:c + 1])
                else:
                    nc.vector.tensor_scalar_mul(
                        oT[c][:, sl], oT[c][:, sl], g1T[:, c:c + 1])
                nc.sync.dma_start(out=outT[c * 128:(c + 1) * 128, sl],
                                  in_=oT[c][:, sl])

    nc.compile()
    return nc


def _get_nc():
    if "nc" not in _cache:
        _cache["nc"] = _build()
    return _cache["nc"]


def _make_in_maps(x, W1, b1, W2, b2, cw1, cb1, cw2, cb2):
    # bf16 + pre-transposed x: (B, N, DIM) -> per-core (DIM, N)
    xb = np.asarray(x, dtype=_BF)
    w1b = np.asarray(W1, dtype=_BF)
    w2b = np.asarray(W2, dtype=_BF)
    cw1b = np.asarray(cw1, dtype=_BF)
    cw2b = np.asarray(cw2, dtype=_BF)
    b1tv = np.ascontiguousarray(
        np.asarray(b1, np.float32).reshape(H, 8, 128).transpose(2, 0, 1)
        .reshape(128, H * 8))
    b2tv = np.ascontiguousarray(
        np.asarray(b2, np.float32).reshape(H, 2, 128).transpose(2, 0, 1)
        .reshape(128, 8))
    cb1v = np.asarray(cb1, np.float32).reshape(SQ, 1)
    cb2tv = np.ascontiguousarray(
        np.asarray(cb2, np.float32).reshape(8, 128).T)

    shared = {
        "w1": w1b, "w2": w2b, "b1t": b1tv, "b2t": b2tv,
        "cw1": cw1b, "cb1t": cb1v, "cw2": cw2b, "cb2t": cb2tv,
    }
    return [dict(shared, xt=np.ascontiguousarray(xb[i].T))
            for i in range(NCORES)]


def kernel(x, W1, b1, W2, b2, cw1, cb1, cw2, cb2):
    from concourse.bass_utils import run_bass_kernel_spmd

    nc = _get_nc()
    in_maps = _make_in_maps(x, W1, b1, W2, b2, cw1, cb1, cw2, cb2)
    res = run_bass_kernel_spmd(nc, in_maps, core_ids=list(range(NCORES)))
    # un-transpose: per-core (DIM, N) -> (N, DIM)
    y = np.stack([res.results[i]["outT"].T for i in range(NCORES)], axis=0)
    return y.astype(np.float32)


# revision 25
# speedup vs baseline: 1.4204x; 1.0080x over previous
"""MultiHeadMlp TRN2 kernel: grouped per-head MLP + SE channel attention.

Full-input contract: kernel(**inputs) takes the complete arrays and returns
the complete output. Internally shards data-parallel over the batch dim
(B=8 -> 8 NeuronCores), builds one SPMD Bass/Tile program, and runs it via
run_bass_kernel_spmd.

Math (per batch element b, all tokens local to one core):
    xh = x.reshape(N, H, D)
    h  = gelu(xh @ W1 + b1)          per head, D=256 -> HID=1024
    o  = h @ W2 + b2                 per head, HID   -> D
    out = concat_heads(o)            (N, C)
    pooled = out.mean(axis=0)        (C,)
    gate = sigmoid(relu(pooled@cw1+cb1)@cw2+cb2)
    y = out * (1 + gate)

Layout strategy: everything on-chip is channel-major ("transposed"):
the host hands the kernel x^T (and un-transposes y^T on the way out), so
W1 [D,HID] / W2 [HID,D] serve directly as matmul lhsT operands, the SE
pool is a free-dim reduction, the gate is a native per-partition scalar
multiply, and the device never transposes anything.
"""

import numpy as np
import ml_dtypes

B = 8
N = 4096
DIM = 1024
H = 4
HD = 256           # head dim
HID = 1024         # per-head hidden
SQ = 64            # squeeze dim
TCH = 512          # tokens per chunk
NCHUNK = N // TCH  # 8
NCORES = 8

_BF = ml_dtypes.bfloat16

_cache = {}


def _build():
    from contextlib import ExitStack

    import concourse.bass as bass
    import concourse.mybir as mybir
    from concourse import bacc
    from concourse.tile import TileContext

    dt = mybir.dt
    bf = dt.bfloat16
    f32 = dt.float32
    Act = mybir.ActivationFunctionType
    Alu = mybir.AluOpType
    Ax = mybir.AxisListType

    nc = bacc.Bacc("TRN2", target_bir_lowering=False, debug=False)

    xt = nc.dram_tensor("xt", [DIM, N], bf, kind="ExternalInput")
    w1 = nc.dram_tensor("w1", [H, HD, HID], bf, kind="ExternalInput")
    w2 = nc.dram_tensor("w2", [H, HID, HD], bf, kind="ExternalInput")
    b1t = nc.dram_tensor("b1t", [128, H * 8], f32, kind="ExternalInput")
    b2t = nc.dram_tensor("b2t", [128, 8], f32, kind="ExternalInput")
    cw1 = nc.dram_tensor("cw1", [DIM, SQ], bf, kind="ExternalInput")
    cb1t = nc.dram_tensor("cb1t", [SQ, 1], f32, kind="ExternalInput")
    cw2 = nc.dram_tensor("cw2", [SQ, DIM], bf, kind="ExternalInput")
    cb2t = nc.dram_tensor("cb2t", [128, 8], f32, kind="ExternalInput")
    outT = nc.dram_tensor("outT", [DIM, N], bf, kind="ExternalOutput")

    with TileContext(nc) as tc, ExitStack() as ctx:
        const = ctx.enter_context(tc.tile_pool(name="const", bufs=1))
        hpool = ctx.enter_context(tc.tile_pool(name="hpool", bufs=2))
        pg1 = ctx.enter_context(tc.tile_pool(name="pg1", bufs=4, space="PSUM"))
        pg2 = ctx.enter_context(tc.tile_pool(name="pg2", bufs=4, space="PSUM"))

        # ---- activation-table + PE-clock warmup (overlaps the load phase) ----
        warm = const.tile([128, 1], f32, name="warm", tag="warm")
        nc.vector.memset(warm, 0.0)
        nc.scalar.activation(out=warm, in_=warm, func=Act.Sigmoid)
        nc.scalar.activation(out=warm, in_=warm, func=Act.Relu)
        nc.scalar.activation(out=warm, in_=warm, func=Act.Gelu)
        # dummy matmuls keep the PE busy through the HAM activity window so
        # the real GEMM stream starts at the warm 2.4 GHz clock
        wmm = const.tile([128, 512], bf, name="wmm", tag="wmm")
        nc.vector.memset(wmm, 0.0)
        for _ in range(12):
            pw = pg1.tile([128, 512], f32, name="p1", tag="p1")
            nc.tensor.matmul(pw, lhsT=wmm[:, 0:128], rhs=wmm,
                             start=True, stop=True)

        # ---- weights + x^T, ordered by first use, all on HWDGE ----
        # consolidated per-head weight tiles: one DMA each
        w1sb = [const.tile([128, 2, HID], bf, name=f"w1sb_{h}",
                           tag=f"w1sb_{h}") for h in range(H)]
        w2sb = [const.tile([128, 8, HD], bf, name=f"w2sb_{h}",
                           tag=f"w2sb_{h}") for h in range(H)]
        xfull = [const.tile([128, N], bf, name=f"xfull_{c}",
                            tag=f"xfull_{c}") for c in range(8)]
        b1sb = const.tile([128, H * 8], f32, name="b1sb", tag="b1sb")
        b2sb = const.tile([128, 8], f32, name="b2sb", tag="b2sb")

        HN = N // 2
        w1r0 = w1[0].rearrange("(k p) n -> p k n", p=128)
        nc.sync.dma_start(out=w1sb[0][:, 0:1, :], in_=w1r0[:, 0:1, :])
        nc.sync.dma_start(out=w1sb[0][:, 1:2, :], in_=w1r0[:, 1:2, :])
        nc.sync.dma_start(out=b1sb, in_=b1t[:, :])
        # first chunk's x slices first for the earliest possible PE start
        nc.sync.dma_start(out=xfull[0][:, :TCH], in_=xt[0:128, :TCH])
        nc.sync.dma_start(out=xfull[1][:, :TCH], in_=xt[128:256, :TCH])
        nc.sync.dma_start(out=xfull[0][:, TCH:HN], in_=xt[0:128, TCH:HN])
        nc.sync.dma_start(out=xfull[1][:, TCH:HN], in_=xt[128:256, TCH:HN])
        nc.sync.dma_start(out=w2sb[0],
                          in_=w2[0].rearrange("(k p) n -> p k n", p=128))
        nc.sync.dma_start(out=b2sb, in_=b2t[:, :])
        for h in range(1, H):
            nc.sync.dma_start(out=w1sb[h],
                              in_=w1[h].rearrange("(k p) n -> p k n", p=128))
            nc.sync.dma_start(out=xfull[2 * h][:, :HN],
                              in_=xt[h * 256:h * 256 + 128, :HN])
            nc.sync.dma_start(out=xfull[2 * h + 1][:, :HN],
                              in_=xt[h * 256 + 128:(h + 1) * 256, :HN])
            nc.sync.dma_start(out=w2sb[h],
                              in_=w2[h].rearrange("(k p) n -> p k n", p=128))
        for c in range(8):
            nc.sync.dma_start(out=xfull[c][:, HN:],
                              in_=xt[c * 128:(c + 1) * 128, HN:])
        cw1sb = const.tile([128, 8, SQ], bf, name="cw1sb", tag="cw1sb")
        nc.sync.dma_start(out=cw1sb,
                          in_=cw1.rearrange("(c p) n -> p c n", p=128))
        cb1sb = const.tile([SQ, 1], f32, name="cb1sb", tag="cb1sb")
        nc.sync.dma_start(out=cb1sb, in_=cb1t[:, :])
        cw2sb = const.tile([SQ, DIM], bf, name="cw2sb", tag="cw2sb")
        nc.sync.dma_start(out=cw2sb, in_=cw2[:, :])
        cb2sb = const.tile([128, 8], f32, name="cb2sb", tag="cb2sb")
        nc.sync.dma_start(out=cb2sb, in_=cb2t[:, :])

        # channel-major out accumulator (persists across whole kernel)
        oT = []
        for c in range(8):
            t = const.tile([128, N], bf, name=f"oT_{c}", tag=f"oT_{c}")
            oT.append(t)
        # per-(chunk, chan-tile) row sums for the SE pool
        prow = const.tile([128, NCHUNK * 8], f32, name="prow", tag="prow")

        # ---- main loop over token chunks ----
        for i in range(NCHUNK):
            t0 = i * TCH
            for h in range(H):
                # GEMM1: h^T[m-tile] = gelu(W1_h^T x^T + b1)
                ht = []
                for m in range(8):
                    p1 = pg1.tile([128, TCH], f32, name="p1", tag="p1")
                    nc.tensor.matmul(
                        p1, lhsT=w1sb[h][:, 0, m * 128:(m + 1) * 128],
                        rhs=xfull[2 * h][:, t0:t0 + TCH],
                        start=True, stop=False)
                    nc.tensor.matmul(
                        p1, lhsT=w1sb[h][:, 1, m * 128:(m + 1) * 128],
                        rhs=xfull[2 * h + 1][:, t0:t0 + TCH],
                        start=False, stop=True)
                    hm = hpool.tile([128, TCH], bf, name=f"ht_{m}",
                                    tag=f"ht_{m}")
                    nc.scalar.activation(
                        out=hm, in_=p1, func=Act.Gelu,
                        bias=b1sb[:, h * 8 + m:h * 8 + m + 1])
                    ht.append(hm)
                # GEMM2: o^T[d-half] = W2_h^T h^T + b2
                for d in range(2):
                    c = h * 2 + d
                    p2 = pg2.tile([128, TCH], f32, name="p2", tag="p2")
                    for k in range(8):
                        nc.tensor.matmul(
                            p2, lhsT=w2sb[h][:, k, d * 128:(d + 1) * 128],
                            rhs=ht[k], start=(k == 0), stop=(k == 7))
                    nc.vector.tensor_scalar(
                        out=oT[c][:, t0:t0 + TCH], in0=p2,
                        scalar1=b2sb[:, c:c + 1],
                        scalar2=0.0, op0=Alu.add, op1=Alu.add,
                        accum_out=prow[:, i * 8 + c:i * 8 + c + 1])

        # prefetch the sigmoid table set while the SE reduction chain runs
        # (the main loop's gelus keep the gelu set resident; without this the
        # 1.3us table load lands between relu and the gate sigmoid)
        nc.scalar.activation(out=warm, in_=warm, func=Act.Sigmoid)

        # ---- SE channel attention on pooled means (all channel-major) ----
        # partial reduction over chunks 0..6 runs as soon as those chunks'
        # row sums exist (overlaps chunk 7 compute); only the final add is
        # on the critical path.
        pooled_part = const.tile([128, 8], f32, name="pooled_part",
                                 tag="pooled_part")
        pooled_raw = const.tile([128, 8], f32, name="pooled_raw",
                                tag="pooled_raw")
        prow3 = prow.rearrange("p (i c) -> p i c", c=8)
        for c in range(8):
            nc.vector.tensor_reduce(
                out=pooled_part[:, c:c + 1], in_=prow3[:, 0:NCHUNK - 1, c],
                axis=Ax.X, op=Alu.add)
        nc.vector.tensor_tensor(out=pooled_raw, in0=pooled_part,
                                in1=prow3[:, NCHUNK - 1, :], op=Alu.add)
        pooledT = const.tile([128, 8], bf, name="pooledT", tag="pooledT")
        nc.vector.tensor_scalar_mul(pooledT, pooled_raw, 1.0 / N)

        pz = pg1.tile([SQ, 1], f32, name="pz", tag="p1")
        for c in range(8):
            nc.tensor.matmul(pz, lhsT=cw1sb[:, c, :], rhs=pooledT[:, c:c + 1],
                             start=(c == 0), stop=(c == 7))
        z1sb = const.tile([SQ, 1], bf, name="z1sb", tag="z1sb")
        nc.scalar.activation(out=z1sb, in_=pz, func=Act.Relu, bias=cb1sb)

        # gate^T = 1 + sigmoid(cw2^T z1 + cb2): one psum tile, one sigmoid
        g1T = const.tile([128, 8], f32, name="g1T", tag="g1T")
        gp8 = pg2.tile([128, 8], f32, name="gp8", tag="p2")
        for c in range(8):
            nc.tensor.matmul(gp8[:, c:c + 1],
                             lhsT=cw2sb[:, c * 128:(c + 1) * 128],
                             rhs=z1sb, start=True, stop=True)
        gadd = const.tile([128, 8], f32, name="gadd", tag="gadd")
        nc.vector.tensor_tensor(out=gadd, in0=gp8, in1=cb2sb, op=Alu.add)
        nc.scalar.activation(out=g1T, in_=gadd, func=Act.Sigmoid)
        nc.vector.tensor_scalar_add(g1T, g1T, 1.0)

        # ---- final scale + store (in-place on oT; DVE with GpSimd assist) ----
        for c in range(8):
            for half in range(2):
                sl = slice(half * 2048, (half + 1) * 2048)
                if (c, half) in ((4, 1), (5, 1), (6, 1), (7, 1)):
                    # ACT takes a few slices in parallel with the DVE stream
                    nc.scalar.activation(
                        out=oT[c][:, sl], in_=oT[c][:, sl],
                        func=Act.Copy, scale=g1T[:, c:c + 1])
                else:
                    nc.vector.tensor_scalar_mul(
                        oT[c][:, sl], oT[c][:, sl], g1T[:, c:c + 1])
                nc.sync.dma_start(out=outT[c * 128:(c + 1) * 128, sl],
                                  in_=oT[c][:, sl])

    nc.compile()
    return nc


def _get_nc():
    if "nc" not in _cache:
        _cache["nc"] = _build()
    return _cache["nc"]


def _make_in_maps(x, W1, b1, W2, b2, cw1, cb1, cw2, cb2):
    # bf16 + pre-transposed x: (B, N, DIM) -> per-core (DIM, N)
    xb = np.asarray(x, dtype=_BF)
    w1b = np.asarray(W1, dtype=_BF)
    w2b = np.asarray(W2, dtype=_BF)
    cw1b = np.asarray(cw1, dtype=_BF)
    cw2b = np.asarray(cw2, dtype=_BF)
    b1tv = np.ascontiguousarray(
        np.asarray(b1, np.float32).reshape(H, 8, 128).transpose(2, 0, 1)
        .reshape(128, H * 8))
    b2tv = np.ascontiguousarray(
        np.asarray(b2, np.float32).reshape(H, 2, 128).transpose(2, 0, 1)
        .reshape(128, 8))
    cb1v = np.asarray(cb1, np.float32).reshape(SQ, 1)
    cb2tv = np.ascontiguousarray(
        np.asarray(cb2, np.float32).reshape(8, 128).T)

    shared = {
        "w1": w1b, "w2": w2b, "b1t": b1tv, "b2t": b2tv,
        "cw1": cw1b, "cb1t": cb1v, "cw2": cw2b, "cb2t": cb2tv,
    }
    return [dict(shared, xt=np.ascontiguousarray(xb[i].T))
            for i in range(NCORES)]


def kernel(x, W1, b1, W2, b2, cw1, cb1, cw2, cb2):
    from concourse.bass_utils import run_bass_kernel_spmd

    nc = _get_nc()
    in_maps = _make_in_maps(x, W1, b1, W2, b2, cw1, cb1, cw2, cb2)
    res = run_bass_kernel_spmd(nc, in_maps, core_ids=list(range(NCORES)))
    # un-transpose: per-core (DIM, N) -> (N, DIM)
    y = np.stack([res.results[i]["outT"].T for i in range(NCORES)], axis=0)
    return y.astype(np.float32)
